# revision 29
# baseline (speedup 1.0000x reference)
"""BitLinear (per-token int8 absmax activation quant + ternary weight quant +
GEMM + bias) for Trainium2, column-parallel over 8 NeuronCores.

v2 strategy ("exact-int" feed): reproduce the reference's activation
quantization ON DEVICE (per-token absmax -> inv scale -> rint via the fp32
magic-constant trick), feed the GEMM the integer values x_int:
  - k-tiles [0, K8):  fp8e4(x_int), contracted 2-at-a-time with
    perf_mode=DoubleRow (2x PE throughput).  Integers |v|<=16 are exact in
    fp8e4m3; larger ones round (this is the ONLY error source).
  - k-tiles [K8, 32): bf16(x_int) -- exact (integers <= 256 in bf16), so
    these tiles contribute ZERO error vs the reference.
The psum (exact integer accumulation in fp32) is scaled by the per-token
scale s_t and bias-added in one fused DVE op at evacuation.

Error is deterministic (fixed inputs): measured on full data per K8.
vs the v1 raw-cast scheme, the bf16 tiles' int8-quantization noise is gone,
so K8 can go slightly higher at equal error.

Sharding: weight/bias column-parallel (out_features/8 = 2048 per core),
activations replicated.  Host prep is layout-only + input-independent weight
ternarization; all input-dependent math runs on device.

Per-chunk device pipeline (128 tokens, x chunk-major transposed [128,kt,tcc]):
  DVE : m1[p,t] = max_k |x[p,k,t]|          (strided tensor_reduce)
  PE  : mT = transpose(m1)                  (tokens on partitions)
  DVE : am_col = max_f mT; s_col = am_col/127
  GPS : m_red = partition_all_reduce(max, m1)   (row/broadcast form)
  DVE : inv_b = 1/(m_red/127 + eps)         (exact DVE reciprocal)
  DVE : qm = x * inv_b                      (stride-0 kt broadcast)
  DVE : xq8 = (qm + MAGIC) - MAGIC -> fp8   (rint, RNE cast)
  DVE : x16 = (qm + MAGIC) - MAGIC -> bf16  (rint, exact cast)
  PE  : 4 psum chains (nf-outer): K8/2 DoubleRow fp8 + (32-K8) bf16 matmuls
  DVE : out = (psum * s_col) + bias         (one scalar_tensor_tensor)
"""

import sys

import numpy as np

if "/opt/trn_rl_repo" not in sys.path:
    sys.path.insert(0, "/opt/trn_rl_repo")

# ---------------------------------------------------------------- constants
B, T, D_IN, D_OUT = 4, 2048, 4096, 16384
NCORES = 8
NTOK = B * T                      # 8192 tokens
OF = D_OUT // NCORES              # 2048 out features per core
P = 128                           # partitions
KT = D_IN // P                    # 32 k-tiles
EPS = 1e-8
THRESH = 0.5
MAGIC = float(np.float32(3 * 2 ** 22))   # fp32 rint trick constant

K8 = 16                           # k-tiles computed in fp8 DoubleRow (even)
TC = 128                          # tokens per chunk


def chunk_schedule(ntok, tc):
    assert ntok % tc == 0
    return [tc] * (ntok // tc)


def pack_x(x2d, tc):
    """Chunk-major transposed layout: for each token chunk, a (128, kt, tcc)
    block stored contiguously per partition."""
    ntok, d_in = x2d.shape
    kt = d_in // P
    out = np.empty((P, ntok * kt), dtype=np.float32)
    t0 = 0
    off = 0
    for tcc in chunk_schedule(ntok, tc):
        blk = x2d[t0:t0 + tcc, :].reshape(tcc, kt, P).transpose(2, 1, 0)
        out[:, off:off + kt * tcc] = blk.reshape(P, kt * tcc)
        t0 += tcc
        off += kt * tcc
    return out


def build_nc(ntok=NTOK, of=OF, tc=TC, k8=K8):
    """Single-core Bass program (SPMD: same program on all cores)."""
    import concourse.mybir as mybir
    from concourse import bacc, bass_isa
    from concourse.masks import make_identity
    from concourse.tile import TileContext

    dt = mybir.dt
    alu = mybir.AluOpType
    kt = KT
    k16 = kt - k8
    nf_t = of // 512               # 4 psum column chunks

    nc = bacc.Bacc("TRN2", target_bir_lowering=False)
    xt = nc.dram_tensor("xt", [P, ntok * kt], dt.float32, kind="ExternalInput")
    if k8:
        w8 = nc.dram_tensor("w8", [P, k8, of], dt.float8e4, kind="ExternalInput")
    if k16:
        w16 = nc.dram_tensor("w16", [P, k16, of], dt.bfloat16, kind="ExternalInput")
    bias = nc.dram_tensor("bias", [1, of], dt.bfloat16, kind="ExternalInput")
    out = nc.dram_tensor("out", [ntok, of], dt.bfloat16, kind="ExternalOutput")

    with TileContext(nc) as tc_:
        with (
            tc_.tile_pool(name="const", bufs=1) as cpool,
            tc_.tile_pool(name="xch", bufs=2) as xpool,
            tc_.tile_pool(name="qm", bufs=2) as qmpool,
            tc_.tile_pool(name="xq", bufs=2) as qpool,
            tc_.tile_pool(name="sc", bufs=2) as spool,
            tc_.tile_pool(name="trp", bufs=1) as tr_pool,
            tc_.tile_pool(name="outs", bufs=2) as opool,
            tc_.tile_pool(name="ps", bufs=3, space="PSUM") as ppool,
            tc_.tile_pool(name="pst", bufs=2, space="PSUM") as tpool,
            tc_.tile_pool(name="pse", bufs=1) as epool,
        ):
            # ---- prologue: x chunks 0/1 win the DMA race, then weights ---
            # (the 16 DMA engines serve transfers roughly in issue order, so
            # the 12MB of weights must queue AFTER the first x chunks or the
            # quant chain starves until ~30us)
            chunk_sizes = chunk_schedule(ntok, tc)
            x_tiles = {}
            w8_sb = None
            for c in range(min(2, len(chunk_sizes))):
                tcc = chunk_sizes[c]
                x_ch = xpool.tile([P, kt * tcc], dt.float32, tag="x",
                                  name=f"x_{c}")
                half = kt * tcc // 2
                xo = c * kt * tcc
                nc.sync.dma_start(x_ch[:, 0:half], xt[:, xo:xo + half])
                nc.sync.dma_start(x_ch[:, half:], xt[:, xo + half:
                                                     xo + kt * tcc])
                x_tiles[c] = x_ch
                if c == 0 and k8:
                    # w8 right after x chunk 0: lands ~17us, just in time for
                    # the first DR matmuls (~16us quant-chain latency)
                    w8_sb = cpool.tile([P, k8, of], dt.float8e4, tag="w8")
                    nc.sync.dma_start(w8_sb[:], w8[:])
            if k16:
                w16_sb = cpool.tile([P, k16, of], dt.bfloat16, tag="w16")
                q16 = max(k16 // 4, 1)
                for ws in range(0, k16, q16):
                    hi = min(ws + q16, k16)
                    nc.sync.dma_start(w16_sb[:, ws:hi, :], w16[:, ws:hi, :])
            bias_bc = cpool.tile([P, of], dt.bfloat16, tag="biasbc")
            nc.scalar.dma_start(bias_bc[:], bias[0:1, :].to_broadcast((P, of)))
            ident = cpool.tile([P, P], dt.float32, tag="ident")
            make_identity(nc, ident)

            # ---- streamed token chunks ----------------------------------
            tok0 = 0
            xoff = 0
            for c, tcc in enumerate(chunk_sizes):
                assert tcc == P
                half = kt * tcc // 2
                if c in x_tiles:
                    x_ch = x_tiles[c]
                else:
                    x_ch = xpool.tile([P, kt * tcc], dt.float32, tag="x",
                                      name=f"x_{c}")
                    nc.sync.dma_start(x_ch[:, 0:half], xt[:, xoff:xoff + half])
                    nc.sync.dma_start(x_ch[:, half:], xt[:, xoff + half:
                                                         xoff + kt * tcc])

                # ---- per-token scale chain ------------------------------
                # m1[p, t] = max_k |x[p, k, t]| / 127, via an ACT Abs pass
                # (idle scalar engine, fused with the 1/127 scale) and a
                # contiguous plain-max binary tree on DVE.  qm doubles as the
                # abs scratch (it is overwritten by the quant multiply later).
                qm = qmpool.tile([P, kt * tcc], dt.float32, tag="qm",
                                 name=f"qm_{c}")
                m1 = spool.tile([P, tcc], dt.float32, tag="m1", name=f"m1_{c}")
                trs = tr_pool.tile([P, kt // 2 * tcc], dt.float32, tag="trs",
                                   name=f"trs_{c}")
                nc.scalar.activation(
                    qm[:, 0:half], x_ch[:, 0:half],
                    mybir.ActivationFunctionType.Abs, scale=1.0 / 127.0)
                nc.scalar.activation(
                    qm[:, half:], x_ch[:, half:],
                    mybir.ActivationFunctionType.Abs, scale=1.0 / 127.0)
                # tree: qm(32) -> trs(16) -> qm(8) -> trs(4) -> qm(2) -> m1
                srcs = [
                    (qm, 0, kt * tcc), (trs, 0, kt // 2 * tcc),
                    (qm, 0, kt // 4 * tcc), (trs, 0, kt // 8 * tcc),
                    (qm, 0, kt // 16 * tcc),
                ]
                for li in range(5):
                    s_t, s0, sz = srcs[li]
                    hsz = sz // 2
                    if li == 4:
                        out_ap = m1[:]
                    else:
                        d_t, d0, _ = srcs[li + 1]
                        out_ap = d_t[:, d0:d0 + hsz]
                    nc.vector.tensor_tensor(
                        out_ap, s_t[:, s0:s0 + hsz],
                        s_t[:, s0 + hsz:s0 + sz], alu.max)
                # column form (tokens on partitions): PE transpose + reduce
                mT = tpool.tile([P, tcc], dt.float32, tag="mT", name=f"mT_{c}")
                nc.tensor.transpose(mT[:], m1[:], ident[:])
                s_col = spool.tile([P, 1], dt.float32, tag="scol",
                                   name=f"scol_{c}")
                nc.vector.tensor_reduce(
                    s_col[:], mT[:], mybir.AxisListType.X, alu.max)
                # row/broadcast form: cross-partition max on gpsimd
                m_red = spool.tile([P, tcc], dt.float32, tag="mred",
                                   name=f"mred_{c}")
                nc.gpsimd.partition_all_reduce(
                    m_red[:], m1[:], channels=P,
                    reduce_op=bass_isa.ReduceOp.max)
                inv_b = spool.tile([P, tcc], dt.float32, tag="invb",
                                   name=f"invb_{c}")
                nc.vector.tensor_scalar(
                    inv_b[:], m_red[:], EPS, None, alu.add)
                nc.vector.reciprocal(inv_b[:], inv_b[:])

                # ---- quantize: qm = x*inv, rint via magic, cast ---------
                inv_ap = inv_b[:].rearrange("p (one t) -> p one t", one=1) \
                    .to_broadcast((P, kt, tcc))
                nc.vector.tensor_tensor(
                    qm[:].rearrange("p (k t) -> p k t", t=tcc),
                    x_ch[:].rearrange("p (k t) -> p k t", t=tcc),
                    inv_ap, alu.mult)
                if k8:
                    xq8 = qpool.tile([P, k8 * tcc], dt.float8e4, tag="xq8",
                                     name=f"xq8_{c}")
                    hq = k8 * tcc // 2
                    nc.vector.tensor_scalar(
                        xq8[:, 0:hq], qm[:, 0:hq], MAGIC, MAGIC,
                        alu.add, alu.subtract)
                    nc.vector.tensor_scalar(
                        xq8[:, hq:], qm[:, hq:k8 * tcc], MAGIC, MAGIC,
                        alu.add, alu.subtract)
                if k16:
                    x16 = qpool.tile([P, k16 * tcc], dt.bfloat16, tag="x16",
                                     name=f"x16_{c}")
                    nc.vector.tensor_scalar(
                        x16[:], qm[:, k8 * tcc:], MAGIC, MAGIC,
                        alu.add, alu.subtract)

                # ---- GEMM: two passes of 2 interleaved psum chains ------
                # (2-way nf interleave keeps DR<->bf16 perf-mode switches to
                # 2 per pass while only needing 2 live psum banks per pass)
                nmm = k8 // 2 + k16
                out_sb = opool.tile([P, of], dt.bfloat16, tag="osb",
                                    name=f"osb_{c}")
                for hp in range(2):
                    psums = [
                        ppool.tile([P, 512], dt.float32, tag=f"ps{j}",
                                   name=f"ps_{c}_{hp}_{j}")
                        for j in range(2)
                    ]
                    for kp in range(k8 // 2):
                        lhs = xq8[:].rearrange("p (k t) -> p k t", t=tcc)[
                            :, 2 * kp:2 * kp + 2, :]
                        for j in range(2):
                            nf = 2 * hp + j
                            nc.tensor.matmul(
                                psums[j], lhs,
                                w8_sb[:, 2 * kp:2 * kp + 2,
                                      nf * 512:(nf + 1) * 512],
                                start=(kp == 0), stop=(kp == nmm - 1),
                                perf_mode=mybir.MatmulPerfMode.DoubleRow,
                            )
                    for kb in range(k16):
                        lhs = x16[:].rearrange("p (k t) -> p k t", t=tcc)[
                            :, kb, :]
                        for j in range(2):
                            nf = 2 * hp + j
                            nc.tensor.matmul(
                                psums[j], lhs,
                                w16_sb[:, kb, nf * 512:(nf + 1) * 512],
                                start=(k8 // 2 + kb == 0),
                                stop=(k8 // 2 + kb == nmm - 1),
                            )
                    # evacuation: ACT (light queue, can read psum) scales by
                    # s_col into a spare psum bank so the GEMM bank frees
                    # promptly; DVE adds bias.  A single fused DVE op instead
                    # makes the psum-free semaphore queue behind the next
                    # chunk's quant ops (~400ns/chunk PE stall).
                    for j in range(2):
                        nf = 2 * hp + j
                        t_ev = epool.tile([P, 512], dt.float32, tag=f"tev{j}",
                                          name=f"tev_{c}_{hp}_{j}")
                        nc.scalar.activation(
                            t_ev[:], psums[j],
                            mybir.ActivationFunctionType.Copy,
                            bias=0.0, scale=s_col[:])
                        nc.vector.tensor_tensor(
                            out_sb[:, nf * 512:(nf + 1) * 512],
                            t_ev[:],
                            bias_bc[:, nf * 512:(nf + 1) * 512],
                            alu.add,
                        )
                nc.scalar.dma_start(out[tok0:tok0 + tcc, :], out_sb[:])
                tok0 += tcc
                xoff += kt * tcc

    nc.finalize()
    return nc


# ------------------------------------------------------------------ host side
def _ternarize_weight(weight):
    """Reproduce the reference's forward weight path exactly (jax fp32 math),
    then cast to the matmul dtypes (snaps the +-1ulp STE noise to ternary)."""
    try:
        import jax
        import jax.numpy as jnp

        with jax.default_device(jax.devices("cpu")[0]):
            w = jnp.asarray(weight)
            w_scale = jnp.mean(jnp.abs(w))
            w_scaled = w / (w_scale + EPS)
            w_q = jnp.sign(w_scaled) * (jnp.abs(w_scaled) > THRESH).astype(w.dtype)
            return np.asarray(w_q).astype(np.float32)
    except Exception:
        w = weight.astype(np.float32)
        w_scale = np.float32(np.mean(np.abs(w), dtype=np.float64))
        w_scaled = w / (w_scale + np.float32(EPS))
        return (np.sign(w_scaled) * (np.abs(w_scaled) > THRESH)).astype(np.float32)


_NC_CACHE = {}
LAST_RESULTS = None


def kernel(x, weight, bias):
    import os

    import ml_dtypes
    from concourse.bass_utils import run_bass_kernel_spmd

    k8 = int(os.environ.get("KERNEL_K8", K8))
    tc = int(os.environ.get("KERNEL_TC", TC))

    key = (k8, tc)
    if key not in _NC_CACHE:
        _NC_CACHE[key] = build_nc(k8=k8, tc=tc)
    nc = _NC_CACHE[key]

    # ---- host prep: layouts + (input-independent) weight ternarization ----
    x2d = np.ascontiguousarray(x.reshape(NTOK, D_IN).astype(np.float32, copy=False))
    x_t = pack_x(x2d, tc)
    w_q = _ternarize_weight(np.asarray(weight))             # (D_OUT, D_IN) fp32
    bias_f = np.asarray(bias).astype(np.float32, copy=False)

    in_maps = []
    for c in range(NCORES):
        w_shard = w_q[c * OF:(c + 1) * OF, :]               # (OF, D_IN)
        wt = np.ascontiguousarray(w_shard.T)                # (D_IN, OF) fp32
        m = {"xt": x_t,
             "bias": bias_f[c * OF:(c + 1) * OF].reshape(1, OF).astype(
                 ml_dtypes.bfloat16)}
        if k8:
            m["w8"] = np.ascontiguousarray(
                wt[:k8 * P].reshape(k8, P, OF).transpose(1, 0, 2)
            ).astype(ml_dtypes.float8_e4m3)
        if k8 < KT:
            m["w16"] = np.ascontiguousarray(
                wt[k8 * P:].reshape(KT - k8, P, OF).transpose(1, 0, 2)
            ).astype(ml_dtypes.bfloat16)
        in_maps.append(m)

    trace = bool(os.environ.get("KERNEL_TRACE"))
    res = run_bass_kernel_spmd(nc, in_maps, core_ids=list(range(NCORES)),
                               trace=trace)
    global LAST_RESULTS
    LAST_RESULTS = res
    outs = [np.asarray(res.results[c]["out"]).astype(np.float32)
            for c in range(NCORES)]
    full = np.concatenate(outs, axis=1)                     # (NTOK, D_OUT)
    return full.reshape(B, T, D_OUT).astype(np.float32, copy=False)
